# revision 13
# baseline (speedup 1.0000x reference)
"""Trainium2 Bass kernel for CellNet GNN message passing (3 phases:
cells->nets, nets->cells, cells->cells; gather + segment-mean + MLP (+LN)).

Contract: kernel(**inputs) takes FULL unsharded numpy inputs and returns the
FULL output tuple (cell_h, net_h), distributing across 8 NeuronCores inside.
"""

import sys

sys.path.insert(0, "/opt/trn_rl_repo")

import numpy as np

from concourse import bacc, bass, mybir, tile
from concourse import bass_utils
from concourse.masks import make_identity
from concourse.library_config import mlp as MLP_LIB

P = 128
D = 256
DH = 128
EPS = 1e-5
NCORES = 8

# dtype for features / matmul operands on device (accumulation stays fp32)
FEAT_DT = mybir.dt.float32
FEAT_NP = np.float32

F32 = mybir.dt.float32
I32 = mybir.dt.int32


def _cdiv(a, b):
    return -(-a // b)


# ---------------------------------------------------------------------------
# Host-side edge preprocessing
# ---------------------------------------------------------------------------

def make_groups(T, gsz=4):
    out = []
    t = 0
    while t < T:
        nt = min(gsz, T - t)
        out.append((t, nt))
        t += nt
    return out


def preprocess_edges(src_g, dst, sh, pad, ncores, half):
    """Partition edges by destination shard and pack into 128-edge chunks
    aligned to 128-destination tiles, split per tile by source table half
    (dma_gather indices are int16, so each gather call addresses < 32768
    rows of the source table).

    src_g : int64 [E] source indices ALREADY remapped to gather-array space.
    dst   : int64 [E] destination node ids in [0, sh*ncores).
    half  : row count of the low half of the gather table.

    Chunk order: per row-group g (of up to 4 dst tiles): all lo-half chunks
    of the group's tiles, then all hi-half chunks.

    Returns (ginfo, TC, idx16_arrs, dst_arrs, rinv_arrs):
      ginfo     : per group dict {c0, nlo, nhi,
                                  tiles: [(t, kl, kh, plo, phi)]}
                  (plo/phi = global chunk positions of the tile's lo/hi runs)
      idx16_arrs: per-core int16 [128, TC*8] (dma_gather wrap layout)
      dst_arrs  : per-core float32 [128, TC] (dst-local in [0,128) or -1)
      rinv_arrs : per-core float32 [128, pad] (1/max(deg,1) replicated)
    """
    T = pad // P
    groups = make_groups(T)
    core = dst // sh
    loc = dst % sh
    tl = loc // P
    dloc = loc % P
    ishi = (src_g >= half).astype(np.int64)

    key = (core * T + tl) * 2 + ishi
    order = np.argsort(key, kind="stable")
    key_s = key[order]
    src_s = (src_g - ishi * half)[order]
    dloc_s = dloc[order]

    counts = np.bincount(key, minlength=ncores * T * 2).reshape(ncores, T, 2)
    kl = _cdiv(counts[:, :, 0], P).max(axis=0)  # [T]
    kh = _cdiv(counts[:, :, 1], P).max(axis=0)

    # chunk positions
    pos = np.zeros((T, 2), np.int64)
    ginfo = []
    c = 0
    for (t0, nt) in groups:
        gi = {"c0": int(c), "nlo": int(kl[t0:t0 + nt].sum()),
              "nhi": int(kh[t0:t0 + nt].sum()), "tiles": []}
        run = c
        for t in range(t0, t0 + nt):
            pos[t, 0] = run
            run += kl[t]
        for t in range(t0, t0 + nt):
            pos[t, 1] = run
            run += kh[t]
        for t in range(t0, t0 + nt):
            gi["tiles"].append((int(t), int(kl[t]), int(kh[t]),
                                int(pos[t, 0]), int(pos[t, 1])))
        c = run
        ginfo.append(gi)
    TC = int(c)

    group_start = np.concatenate([[0], np.cumsum(counts.transpose(0, 1, 2)
                                                 .ravel())])[:-1]
    rank = np.arange(len(dst)) - group_start[key_s]

    idx16_arrs, dst_arrs, rinv_arrs = [], [], []
    deg = np.bincount(dst, minlength=sh * ncores).astype(np.float32)
    inv = 1.0 / np.maximum(deg, 1.0)
    pos_flat = pos.ravel()  # [(t,h)] -> chunk pos
    for cc in range(ncores):
        sel = (key_s // (2 * T)) == cc
        th = key_s[sel] % (2 * T)          # t*2 + h
        r_c = rank[sel]
        col = pos_flat[th] + r_c // P
        row = r_c % P
        idx_flat = np.zeros((TC * P,), np.int32)
        da = np.full((P, TC), -1.0, np.float32)
        idx_flat[col * P + row] = src_s[sel]
        da[row, col] = dloc_s[sel]
        # dma_gather wrap: edge j -> [16*rep + j%16, j//16]
        i16 = np.ascontiguousarray(
            np.tile(idx_flat.reshape(TC * 8, 16).T.astype(np.int16), (8, 1)))
        idx16_arrs.append(i16)
        dst_arrs.append(da)
        rv = np.zeros((pad,), np.float32)
        rv[:sh] = inv[cc * sh:(cc + 1) * sh]
        rinv_arrs.append(np.ascontiguousarray(np.broadcast_to(rv, (P, pad))))
    return ginfo, TC, idx16_arrs, dst_arrs, rinv_arrs


def _remap(idx, sh, pad):
    """global node id -> index in padded-concat (AllGather) layout."""
    return (idx // sh) * pad + (idx % sh)


# ---------------------------------------------------------------------------
# Device program
# ---------------------------------------------------------------------------

def _bcast_chunks(ap_2d, extra, axis_inner):
    """Append / interleave a broadcast dim on a 2-D SBUF AP.

    axis_inner=True : [p, k] -> [p, k, B(extra)]  (value repeated along new
                      innermost dim)
    axis_inner=False: [p, n] -> [p, B(extra), n]
    """
    base = ap_2d
    if axis_inner:
        new = [list(base.ap[0]), list(base.ap[1]), [0, extra]]
    else:
        new = [list(base.ap[0]), [0, extra], list(base.ap[1])]
    return bass.AP(base.tensor, base.offset, new)


class Builder:
    def __init__(self, plan):
        self.plan = plan
        nc = bacc.Bacc(
            "TRN2",
            target_bir_lowering=False,
            debug=False,
            num_devices=plan["ncores"],
        )
        self.nc = nc
        self.pad = plan["pad"]          # nodes per core per class (mult of 128)
        self.T = self.pad // P          # dst tiles per class
        self.gpad = plan["ncores"] * self.pad

        # ---- I/O tensors -------------------------------------------------
        dram = nc.dram_tensor
        self.t_cell_lo = dram("cell_lo_gather", [self.gpad, D], FEAT_DT,
                              kind="ExternalInput")
        self.t_netT = dram("netT", [D, self.pad], FEAT_DT, kind="ExternalInput")
        self.t_cellT = dram("cellT", [D, 2 * self.pad], FEAT_DT,
                            kind="ExternalInput")
        self.t_w = {}
        for m in ("c2n", "n2c", "c2c"):
            self.t_w[m, "w1"] = dram(f"{m}_w1", [2 * D, D], FEAT_DT,
                                     kind="ExternalInput")
            self.t_w[m, "w2"] = dram(f"{m}_w2", [D, D], FEAT_DT,
                                     kind="ExternalInput")
            self.t_w[m, "b1"] = dram(f"{m}_b1", [P, 2], F32,
                                     kind="ExternalInput")
            self.t_w[m, "b2"] = dram(f"{m}_b2", [P, 2], F32,
                                     kind="ExternalInput")
        self.t_edge = {}
        for a in ("c2n", "n2c", "c2c"):
            TC = max(plan["TC"][a], 1)
            self.t_edge[a, "idx"] = dram(f"{a}_idx", [P, TC * 8],
                                         mybir.dt.int16, kind="ExternalInput")
            self.t_edge[a, "dst"] = dram(f"{a}_dst", [P, TC], F32,
                                         kind="ExternalInput")
            self.t_edge[a, "rinv"] = dram(f"{a}_rinv", [P, self.pad], F32,
                                          kind="ExternalInput")
        if not plan["net_ln_trivial"]:
            self.t_net_g = dram("net_ln_g_b", [P, 2 * D], F32,
                                kind="ExternalInput")
        if not plan["cell_ln_trivial"]:
            self.t_cell_g = dram("cell_ln_g_b", [P, 2 * D], F32,
                                 kind="ExternalInput")

        self.t_net_out = dram("net_out", [self.pad, D], F32,
                              kind="ExternalOutput")
        self.t_cell_out = dram("cell_out", [2 * self.pad, D], F32,
                               kind="ExternalOutput")

        # internal DRAM
        self.t_net_rm = dram("net_rm", [self.pad, D], FEAT_DT, kind="Internal")
        self.t_net_ag = dram("net_ag", [self.gpad, D], FEAT_DT,
                             kind="Internal", addr_space="Shared")
        self.t_cell_rm = dram("cell_rm", [self.pad, D], FEAT_DT,
                              kind="Internal")
        self.t_cell_ag = dram("cell_ag", [self.gpad, D], FEAT_DT,
                              kind="Internal", addr_space="Shared")
        self.t_cell2T = dram("cell2T", [D, 2 * self.pad], FEAT_DT,
                             kind="Internal")

    # -- helpers -----------------------------------------------------------

    def groups(self):
        return make_groups(self.T)

    def load_weights(self, wpool):
        nc = self.nc
        self.w_sb = {}
        self.wpool = wpool
        for m in ("c2n", "n2c", "c2c"):
            w1 = self.wpool.tile([P, 4 * D], FEAT_DT, name=f"w1_{m}")
            for c in range(4):
                nc.sync.dma_start(out=w1[:, c * D:(c + 1) * D],
                                  in_=self.t_w[m, "w1"][c * P:(c + 1) * P, :])
            w2 = self.wpool.tile([P, 2 * D], FEAT_DT, name=f"w2_{m}")
            for c in range(2):
                nc.sync.dma_start(out=w2[:, c * D:(c + 1) * D],
                                  in_=self.t_w[m, "w2"][c * P:(c + 1) * P, :])
            b1 = self.wpool.tile([P, 2], F32, name=f"b1_{m}")
            nc.sync.dma_start(out=b1[:], in_=self.t_w[m, "b1"][:])
            b2 = self.wpool.tile([P, 2], F32, name=f"b2_{m}")
            nc.sync.dma_start(out=b2[:], in_=self.t_w[m, "b2"][:])
            self.w_sb[m] = (w1, w2, b1, b2)

        it_i32 = self.wpool.tile([P, P], I32, name="iota_i")
        nc.gpsimd.iota(it_i32[:], pattern=[[1, P]], base=0,
                       channel_multiplier=0)
        self.iota_f = self.wpool.tile([P, P], FEAT_DT, name="iota_f")
        nc.vector.tensor_copy(out=self.iota_f[:], in_=it_i32[:])
        self.ident = self.wpool.tile([P, P], F32, name="ident")
        make_identity(nc, self.ident[:])
        self.eps_t = self.wpool.tile([P, 1], F32, name="eps_t")
        nc.vector.memset(self.eps_t[:], EPS)
        self.lngb = {}
        if not self.plan["net_ln_trivial"]:
            g = self.wpool.tile([P, 2 * D], F32, name="net_gb")
            nc.sync.dma_start(out=g[:], in_=self.t_net_g[:])
            self.lngb["net"] = g
        if not self.plan["cell_ln_trivial"]:
            g = self.wpool.tile([P, 2 * D], F32, name="cell_gb")
            nc.sync.dma_start(out=g[:], in_=self.t_cell_g[:])
            self.lngb["cell"] = g

    def load_edge_meta(self, tc, a):
        """Load idx/dst/rinv for aggregation `a` into SBUF."""
        nc = self.nc
        TC = max(self.plan["TC"][a], 1)
        idx = self.epool.tile([P, TC * 8], mybir.dt.int16, tag="idx",
                              bufs=2, name=f"idx_{a}")
        nc.sync.dma_start(out=idx[:], in_=self.t_edge[a, "idx"][:])
        dstv = self.epool.tile([P, TC], F32, tag="dst", bufs=2,
                               name=f"dst_{a}")
        nc.sync.dma_start(out=dstv[:], in_=self.t_edge[a, "dst"][:])
        rinv = self.epool.tile([P, self.pad], F32, tag="rinv", bufs=1,
                               name=f"rinv_{a}")
        nc.sync.dma_start(out=rinv[:], in_=self.t_edge[a, "rinv"][:])
        return idx, dstv, rinv

    def aggregate_group(self, a, meta, src_dram, gi, nt):
        """Compute degree-normalized messages (feature-major) for the dst
        tiles of group `gi`. Returns (msg_lo, msg_hi) SBUF [128, nt*128]."""
        nc = self.nc
        idx, dstv, rinv = meta
        c0, nlo, nhi = gi["c0"], gi["nlo"], gi["nhi"]
        kg = nlo + nhi
        half = self.plan["half"]
        R = nt * P

        msg_lo = self.msgsb.tile([P, R], FEAT_DT, tag="msg_lo", name="msg_lo")
        msg_hi = self.msgsb.tile([P, R], FEAT_DT, tag="msg_hi", name="msg_hi")
        if kg == 0:
            nc.vector.memset(msg_lo[:], 0.0)
            nc.vector.memset(msg_hi[:], 0.0)
            return msg_lo, msg_hi

        # SWDGE descriptor carveout fits ~64 descs/lane; one gather call of
        # c chunks emits c*8+2 descs/lane -> cap calls at 7 chunks (896 idxs).
        GMAX = 7
        F = self.fpool.tile([P, kg, D], FEAT_DT, tag="gather", name="Fg")
        for (base, cnt, src) in ((0, nlo, src_dram[:]),
                                 (nlo, nhi, src_dram[half:, :])):
            for s in range(base, base + cnt, GMAX):
                nn = min(GMAX, base + cnt - s)
                n = nn * P
                nc.gpsimd.dma_gather(F[:, s:s + nn, :], src,
                                     idx[:, (c0 + s) * 8:(c0 + s + nn) * 8],
                                     n, n, D)

        ps_lo = self.mpsum.tile([P, R], F32, tag="ps_lo", name="ps_lo")
        ps_hi = self.mpsum.tile([P, R], F32, tag="ps_hi", name="ps_hi")

        for ti, (t, kl, kh, plo, phi) in enumerate(gi["tiles"]):
            k = kl + kh
            osl = slice(ti * P, (ti + 1) * P)
            if k == 0:
                nc.vector.memset(msg_lo[:, osl], 0.0)
                nc.vector.memset(msg_hi[:, osl], 0.0)
                continue
            done = 0
            for (p0, kk) in ((plo, kl), (phi, kh)):
                if kk == 0:
                    continue
                S = self.spool.tile([P, kk * P], FEAT_DT, tag="sel", name="S")
                in0 = _bcast_chunks(self.iota_f[:], kk, axis_inner=False)
                in1 = _bcast_chunks(dstv[:, p0:p0 + kk], P, axis_inner=True)
                nc.vector.tensor_tensor(out=S[:], in0=in0, in1=in1,
                                        op=mybir.AluOpType.is_equal)
                for j in range(kk):
                    rel = p0 - c0 + j
                    nc.tensor.matmul(out=ps_lo[:, osl],
                                     lhsT=F[:, rel, 0:DH],
                                     rhs=S[:, j * P:(j + 1) * P],
                                     start=(done + j == 0),
                                     stop=(done + j == k - 1))
                    nc.tensor.matmul(out=ps_hi[:, osl],
                                     lhsT=F[:, rel, DH:D],
                                     rhs=S[:, j * P:(j + 1) * P],
                                     start=(done + j == 0),
                                     stop=(done + j == k - 1))
                done += kk
            # normalize by 1/deg while copying PSUM -> SBUF
            rs = rinv[:, t * P:(t + 1) * P]
            nc.vector.tensor_tensor(out=msg_lo[:, osl], in0=ps_lo[:, osl],
                                    in1=rs, op=mybir.AluOpType.mult)
            nc.vector.tensor_tensor(out=msg_hi[:, osl], in0=ps_hi[:, osl],
                                    in1=rs, op=mybir.AluOpType.mult)
        return msg_lo, msg_hi

    def mlp(self, m, h_lo, h_hi, msg, R):
        """Feature-major 2-layer MLP + residual.

        h_lo/h_hi: [128, R] SBUF tiles (input features, chunks 0/1)
        msg: None or (msg_lo, msg_hi)
        Returns (z0, z1) SBUF tiles [128, R] (feat chunks of the residual sum).
        """
        nc = self.nc
        w1, w2, b1, b2 = self.w_sb[m]
        chunks = [h_lo, h_hi] + ([msg[0], msg[1]] if msg is not None else [])
        ys = []
        for mi in range(2):
            o1 = self.mlppsum.tile([P, R], F32, tag="mm", name=f"o1_{mi}")
            for c, rt in enumerate(chunks):
                nc.tensor.matmul(out=o1[:],
                                 lhsT=w1[:, c * D + mi * DH:c * D + (mi + 1) * DH],
                                 rhs=rt[:],
                                 start=(c == 0), stop=(c == len(chunks) - 1))
            y = self.ypool.tile([P, R], FEAT_DT, tag="y1", name=f"y1_{mi}")
            nc.scalar.activation(out=y[:], in_=o1[:],
                                 func=mybir.ActivationFunctionType.Relu,
                                 bias=b1[:, mi:mi + 1], scale=1.0)
            ys.append(y)
        zs = []
        for mi in range(2):
            o2 = self.mlppsum.tile([P, R], F32, tag="mm", name=f"o2_{mi}")
            for c in range(2):
                nc.tensor.matmul(out=o2[:],
                                 lhsT=w2[:, c * D + mi * DH:c * D + (mi + 1) * DH],
                                 rhs=ys[c][:],
                                 start=(c == 0), stop=(c == 1))
            # bias + residual: z = (o2 + b2) + h
            zb = self.zpool.tile([P, R], FEAT_DT, tag="zb", bufs=2,
                                 name=f"zb_{mi}")
            nc.scalar.activation(out=zb[:], in_=o2[:],
                                 func=mybir.ActivationFunctionType.Identity,
                                 bias=b2[:, mi:mi + 1], scale=1.0)
            z = self.zpool.tile([P, R], FEAT_DT, tag="z", bufs=4,
                                 name=f"zr_{mi}")
            nc.vector.tensor_tensor(out=z[:], in0=zb[:],
                                    in1=(h_lo if mi == 0 else h_hi)[:],
                                    op=mybir.AluOpType.add)
            zs.append(z)
        return zs

    def row_major_tiles(self, zs, nt):
        """Transpose feature-major z (2x [128, nt*128]) into row-major
        [128, 256] tiles; yields (rt_index, row_tile)."""
        nc = self.nc
        for rt in range(nt):
            tp0 = self.tpsum.tile([P, P], FEAT_DT, tag="tp", name="tp0")
            tp1 = self.tpsum.tile([P, P], FEAT_DT, tag="tp", name="tp1")
            nc.tensor.transpose(out=tp0[:], in_=zs[0][:, rt * P:(rt + 1) * P],
                                identity=self.ident[:])
            nc.tensor.transpose(out=tp1[:], in_=zs[1][:, rt * P:(rt + 1) * P],
                                identity=self.ident[:])
            row = self.rowpool.tile([P, D], FEAT_DT, tag="row", name="row")
            nc.any.tensor_copy(out=row[:, :DH], in_=tp0[:])
            nc.any.tensor_copy(out=row[:, DH:], in_=tp1[:])
            yield rt, row

    def layernorm_row(self, row, which):
        """LayerNorm along free dim of row-major [128, 256] tile -> f32."""
        nc = self.nc
        st = self.lnpool.tile([P, 6], F32, tag="ln6", name="st6")
        nc.vector.bn_stats(out=st[:], in_=row[:])
        mv = self.lnpool.tile([P, 2], F32, tag="ln2", name="mv")
        nc.vector.bn_aggr(out=mv[:], in_=st[:])
        std = self.lnpool.tile([P, 1], F32, tag="ln1a", name="std")
        nc.scalar.activation(out=std[:], in_=mv[:, 1:2],
                             func=mybir.ActivationFunctionType.Sqrt,
                             bias=self.eps_t[:, 0:1], scale=1.0)
        rstd = self.lnpool.tile([P, 1], F32, tag="ln1b", name="rstd")
        nc.vector.reciprocal(out=rstd[:], in_=std[:])
        out = self.orowpool.tile([P, D], F32, tag="orow", name="orow")
        nc.vector.tensor_scalar(out[:], row[:], mv[:, 0:1], rstd[:, 0:1],
                                mybir.AluOpType.subtract,
                                mybir.AluOpType.mult)
        gb = self.lngb.get(which)
        if gb is not None:
            nc.vector.tensor_tensor(out=out[:], in0=out[:], in1=gb[:, :D],
                                    op=mybir.AluOpType.mult)
            nc.vector.tensor_tensor(out=out[:], in0=out[:], in1=gb[:, D:],
                                    op=mybir.AluOpType.add)
        return out

    # -- full program --------------------------------------------------------

    def build(self):
        nc = self.nc
        plan = self.plan
        rg = [list(range(plan["ncores"]))]
        with tile.TileContext(nc) as tc:
            with tc.tile_pool(name="edges", bufs=1) as epool, \
                 tc.tile_pool(name="gather", bufs=2) as fpool, \
                 tc.tile_pool(name="sel", bufs=4) as spool, \
                 tc.tile_pool(name="msgps", bufs=1, space="PSUM") as mpsum, \
                 tc.tile_pool(name="mlpps", bufs=4, space="PSUM") as mlppsum, \
                 tc.tile_pool(name="tpps", bufs=2, space="PSUM") as tpsum, \
                 tc.tile_pool(name="msgsb", bufs=2) as msgsb, \
                 tc.tile_pool(name="hpool", bufs=2) as hpool, \
                 tc.tile_pool(name="ypool", bufs=3) as ypool, \
                 tc.tile_pool(name="zpool", bufs=3) as zpool, \
                 tc.tile_pool(name="rows", bufs=4) as rowpool, \
                 tc.tile_pool(name="orows", bufs=4) as orowpool, \
                 tc.tile_pool(name="ln", bufs=4) as lnpool, \
                 tc.tile_pool(name="weights", bufs=1) as wpool:
                self.epool = epool
                self.fpool = fpool
                self.spool = spool
                self.mpsum = mpsum
                self.mlppsum = mlppsum
                self.tpsum = tpsum
                self.msgsb = msgsb
                self.hpool = hpool
                self.ypool = ypool
                self.zpool = zpool
                self.rowpool = rowpool
                self.orowpool = orowpool
                self.lnpool = lnpool

                self.load_weights(wpool)
                nc.gpsimd.load_library(MLP_LIB)

                # ---------------- Phase 1: cells -> nets -------------------
                meta = self.load_edge_meta(tc, "c2n")
                for gi, (t0, nt) in zip(self.plan["ginfo"]["c2n"],
                                        self.groups()):
                    R = nt * P
                    col = t0 * P
                    msg = self.aggregate_group("c2n", meta, self.t_cell_lo,
                                               gi, nt)
                    h_lo = hpool.tile([P, R], FEAT_DT, tag="h_lo", name="h_lo")
                    h_hi = hpool.tile([P, R], FEAT_DT, tag="h_hi", name="h_hi")
                    nc.sync.dma_start(out=h_lo[:],
                                      in_=self.t_netT[0:DH, col:col + R])
                    nc.sync.dma_start(out=h_hi[:],
                                      in_=self.t_netT[DH:D, col:col + R])
                    zs = self.mlp("c2n", h_lo, h_hi, msg, R)
                    for rt, row in self.row_major_tiles(zs, nt):
                        orow = self.layernorm_row(row, "net")
                        r0 = col + rt * P
                        nc.sync.dma_start(out=self.t_net_out[r0:r0 + P, :],
                                          in_=orow[:])
                        nc.sync.dma_start(out=self.t_net_rm[r0:r0 + P, :],
                                          in_=orow[:])

                nc.gpsimd.collective_compute(
                    "AllGather", mybir.AluOpType.bypass, replica_groups=rg,
                    ins=[self.t_net_rm.ap()[:]], outs=[self.t_net_ag.ap()[:]])

                # ---------------- Phase 2: nets -> cells -------------------
                meta = self.load_edge_meta(tc, "n2c")
                # high cells first (no aggregation; overlaps the AllGather)
                for (t0, nt) in self.groups():
                    R = nt * P
                    col = self.pad + t0 * P
                    h_lo = hpool.tile([P, R], FEAT_DT, tag="h_lo", name="h_lo")
                    h_hi = hpool.tile([P, R], FEAT_DT, tag="h_hi", name="h_hi")
                    nc.sync.dma_start(out=h_lo[:],
                                      in_=self.t_cellT[0:DH, col:col + R])
                    nc.sync.dma_start(out=h_hi[:],
                                      in_=self.t_cellT[DH:D, col:col + R])
                    zs = self.mlp("n2c", h_lo, h_hi, None, R)
                    nc.sync.dma_start(out=self.t_cell2T[0:DH, col:col + R],
                                      in_=zs[0][:])
                    nc.sync.dma_start(out=self.t_cell2T[DH:D, col:col + R],
                                      in_=zs[1][:])
                # low cells
                for gi, (t0, nt) in zip(self.plan["ginfo"]["n2c"],
                                        self.groups()):
                    R = nt * P
                    col = t0 * P
                    msg = self.aggregate_group("n2c", meta,
                                               self.t_net_ag, gi, nt)
                    h_lo = hpool.tile([P, R], FEAT_DT, tag="h_lo", name="h_lo")
                    h_hi = hpool.tile([P, R], FEAT_DT, tag="h_hi", name="h_hi")
                    nc.sync.dma_start(out=h_lo[:],
                                      in_=self.t_cellT[0:DH, col:col + R])
                    nc.sync.dma_start(out=h_hi[:],
                                      in_=self.t_cellT[DH:D, col:col + R])
                    zs = self.mlp("n2c", h_lo, h_hi, msg, R)
                    nc.sync.dma_start(out=self.t_cell2T[0:DH, col:col + R],
                                      in_=zs[0][:])
                    nc.sync.dma_start(out=self.t_cell2T[DH:D, col:col + R],
                                      in_=zs[1][:])
                    for rt, row in self.row_major_tiles(zs, nt):
                        r0 = col + rt * P
                        nc.sync.dma_start(out=self.t_cell_rm[r0:r0 + P, :],
                                          in_=row[:])

                nc.gpsimd.collective_compute(
                    "AllGather", mybir.AluOpType.bypass, replica_groups=rg,
                    ins=[self.t_cell_rm.ap()[:]],
                    outs=[self.t_cell_ag.ap()[:]])

                # ---------------- Phase 3: cells -> cells ------------------
                meta = self.load_edge_meta(tc, "c2c")
                # high cells (no aggregation; overlaps the AllGather)
                for (t0, nt) in self.groups():
                    R = nt * P
                    col = self.pad + t0 * P
                    h_lo = hpool.tile([P, R], FEAT_DT, tag="h_lo", name="h_lo")
                    h_hi = hpool.tile([P, R], FEAT_DT, tag="h_hi", name="h_hi")
                    nc.sync.dma_start(out=h_lo[:],
                                      in_=self.t_cell2T[0:DH, col:col + R])
                    nc.sync.dma_start(out=h_hi[:],
                                      in_=self.t_cell2T[DH:D, col:col + R])
                    zs = self.mlp("c2c", h_lo, h_hi, None, R)
                    for rt, row in self.row_major_tiles(zs, nt):
                        orow = self.layernorm_row(row, "cell")
                        r0 = col + rt * P
                        nc.sync.dma_start(out=self.t_cell_out[r0:r0 + P, :],
                                          in_=orow[:])
                # low cells
                for gi, (t0, nt) in zip(self.plan["ginfo"]["c2c"],
                                        self.groups()):
                    R = nt * P
                    col = t0 * P
                    msg = self.aggregate_group("c2c", meta,
                                               self.t_cell_ag, gi, nt)
                    h_lo = hpool.tile([P, R], FEAT_DT, tag="h_lo", name="h_lo")
                    h_hi = hpool.tile([P, R], FEAT_DT, tag="h_hi", name="h_hi")
                    nc.sync.dma_start(out=h_lo[:],
                                      in_=self.t_cell2T[0:DH, col:col + R])
                    nc.sync.dma_start(out=h_hi[:],
                                      in_=self.t_cell2T[DH:D, col:col + R])
                    zs = self.mlp("c2c", h_lo, h_hi, msg, R)
                    for rt, row in self.row_major_tiles(zs, nt):
                        orow = self.layernorm_row(row, "cell")
                        r0 = col + rt * P
                        nc.sync.dma_start(out=self.t_cell_out[r0:r0 + P, :],
                                          in_=orow[:])

        nc.compile()
        return nc


# ---------------------------------------------------------------------------
# Host orchestration
# ---------------------------------------------------------------------------

_CACHE = {}


def prepare(inputs, n_cell, n_net, ncores=NCORES):
    sh = n_net // ncores            # nodes per core per class
    pad = _cdiv(sh, P) * P
    assert n_cell == 2 * n_net

    cell_h = np.asarray(inputs["cell_h"], np.float32)
    net_h = np.asarray(inputs["net_h"], np.float32)

    e_c2n = np.asarray(inputs["cell_to_net_edge_index"]).astype(np.int64)
    e_n2c = np.asarray(inputs["net_to_cell_edge_index"]).astype(np.int64)
    e_c2c = np.asarray(inputs["cell_to_cell_edge_index"]).astype(np.int64)

    half = (ncores // 2) * pad
    plan = {"ncores": ncores, "pad": pad, "half": half, "ginfo": {},
            "TC": {}}
    edge_data = {}
    for name, (src, dst) in (("c2n", (e_c2n[0], e_c2n[1])),
                             ("n2c", (e_n2c[0], e_n2c[1])),
                             ("c2c", (e_c2c[0], e_c2c[1]))):
        gf, TC, ia, da, rv = preprocess_edges(_remap(src, sh, pad), dst, sh,
                                              pad, ncores, half)
        plan["ginfo"][name] = gf
        plan["TC"][name] = TC
        edge_data[name] = (ia, da, rv)

    net_g = np.asarray(inputs["net_ln_g"], np.float32)
    net_b = np.asarray(inputs["net_ln_b"], np.float32)
    cell_g = np.asarray(inputs["cell_ln_g"], np.float32)
    cell_b = np.asarray(inputs["cell_ln_b"], np.float32)
    plan["net_ln_trivial"] = bool(np.all(net_g == 1.0) and np.all(net_b == 0.0))
    plan["cell_ln_trivial"] = bool(np.all(cell_g == 1.0)
                                   and np.all(cell_b == 0.0))

    # replicated gather source: original low cells, padded-concat layout
    gpad = ncores * pad
    cell_lo = np.zeros((gpad, D), FEAT_NP)
    for c in range(ncores):
        cell_lo[c * pad:c * pad + sh] = cell_h[c * sh:(c + 1) * sh]

    in_maps = []
    for c in range(ncores):
        m = {"cell_lo_gather": cell_lo}
        netT = np.zeros((D, pad), FEAT_NP)
        netT[:, :sh] = net_h[c * sh:(c + 1) * sh].T
        m["netT"] = netT
        cellT = np.zeros((D, 2 * pad), FEAT_NP)
        cellT[:, :sh] = cell_h[c * sh:(c + 1) * sh].T
        cellT[:, pad:pad + sh] = cell_h[n_net + c * sh:n_net + (c + 1) * sh].T
        m["cellT"] = cellT
        for mm in ("c2n", "n2c", "c2c"):
            m[f"{mm}_w1"] = np.asarray(inputs[f"{mm}_w1"], FEAT_NP)
            m[f"{mm}_w2"] = np.asarray(inputs[f"{mm}_w2"], FEAT_NP)
            m[f"{mm}_b1"] = np.ascontiguousarray(
                np.asarray(inputs[f"{mm}_b1"], np.float32).reshape(2, P).T)
            m[f"{mm}_b2"] = np.ascontiguousarray(
                np.asarray(inputs[f"{mm}_b2"], np.float32).reshape(2, P).T)
        for a in ("c2n", "n2c", "c2c"):
            ia, da, rv = edge_data[a]
            m[f"{a}_idx"] = (ia[c] if ia[c].shape[1]
                             else np.zeros((P, 8), np.int16))
            m[f"{a}_dst"] = (da[c] if da[c].shape[1]
                             else np.full((P, 1), -1.0, np.float32))
            m[f"{a}_rinv"] = rv[c]
        if not plan["net_ln_trivial"]:
            m["net_ln_g_b"] = np.ascontiguousarray(np.broadcast_to(
                np.concatenate([net_g, net_b]), (P, 2 * D)).astype(np.float32))
        if not plan["cell_ln_trivial"]:
            m["cell_ln_g_b"] = np.ascontiguousarray(np.broadcast_to(
                np.concatenate([cell_g, cell_b]), (P, 2 * D)).astype(np.float32))
        in_maps.append(m)
    return plan, in_maps, sh, pad


def assemble(results, n_cell, n_net, sh, pad, ncores=NCORES):
    cell = np.empty((n_cell, D), np.float32)
    net = np.empty((n_net, D), np.float32)
    for c in range(ncores):
        co = results[c]["cell_out"]
        cell[c * sh:(c + 1) * sh] = co[:sh]
        cell[n_net + c * sh:n_net + (c + 1) * sh] = co[pad:pad + sh]
        net[c * sh:(c + 1) * sh] = results[c]["net_out"][:sh]
    return cell, net


def get_program(plan):
    def gkey(a):
        return tuple((g["c0"], g["nlo"], g["nhi"], tuple(g["tiles"]))
                     for g in plan["ginfo"][a])
    key = (tuple(gkey(a) for a in ("c2n", "n2c", "c2c")),
           plan["pad"], plan["ncores"], plan["net_ln_trivial"],
           plan["cell_ln_trivial"])
    if key not in _CACHE:
        _CACHE[key] = Builder(plan).build()
    return _CACHE[key]


def kernel(**inputs):
    n_cell = inputs["cell_h"].shape[0]
    n_net = inputs["net_h"].shape[0]
    plan, in_maps, sh, pad = prepare(inputs, n_cell, n_net)
    nc = get_program(plan)
    res = bass_utils.run_bass_kernel_spmd(nc, in_maps,
                                          core_ids=list(range(NCORES)))
    return assemble(res.results, n_cell, n_net, sh, pad)


# revision 21
# speedup vs baseline: 52.6053x; 52.6053x over previous
"""Trainium2 Bass kernel for CellNet GNN message passing (3 phases:
cells->nets, nets->cells, cells->cells; gather + segment-mean + MLP (+LN)).

Contract: kernel(**inputs) takes FULL unsharded numpy inputs and returns the
FULL output tuple (cell_h, net_h), distributing across 8 NeuronCores inside.
"""

import sys

sys.path.insert(0, "/opt/trn_rl_repo")

import numpy as np

from concourse import bacc, bass, mybir, tile
from concourse import bass_utils
from concourse.masks import make_identity
from concourse.library_config import mlp as MLP_LIB

P = 128
D = 256
DH = 128
EPS = 1e-5
NCORES = 8
NQ = 4  # SWDGE queues used for gather rotation

import ml_dtypes

# dtype for features / matmul operands on device (accumulation stays fp32)
FEAT_DT = mybir.dt.bfloat16
FEAT_NP = ml_dtypes.bfloat16

F32 = mybir.dt.float32
I32 = mybir.dt.int32


def _cdiv(a, b):
    return -(-a // b)


# ---------------------------------------------------------------------------
# Host-side edge preprocessing
# ---------------------------------------------------------------------------

def make_groups(T, gsz=4):
    out = []
    t = 0
    while t < T:
        nt = min(gsz, T - t)
        out.append((t, nt))
        t += nt
    return out


def preprocess_edges(src_g, dst, sh, pad, ncores, half):
    """Partition edges by destination shard and pack into 128-edge chunks
    aligned to 128-destination tiles, split per tile by source table half
    (dma_gather indices are int16, so each gather call addresses < 32768
    rows of the source table).

    src_g : int64 [E] source indices ALREADY remapped to gather-array space.
    dst   : int64 [E] destination node ids in [0, sh*ncores).
    half  : row count of the low half of the gather table.

    Chunk order: per row-group g (of up to 4 dst tiles): all lo-half chunks
    of the group's tiles, then all hi-half chunks.

    Returns (ginfo, TC, idx16_arrs, dst_arrs, rinv_arrs):
      ginfo     : per group dict {c0, nlo, nhi,
                                  tiles: [(t, kl, kh, plo, phi)]}
                  (plo/phi = global chunk positions of the tile's lo/hi runs)
      idx16_arrs: per-core int16 [128, TC*8] (dma_gather wrap layout)
      dst_arrs  : per-core bfloat16 [128, TC] (dst-local in [0,128) or -1)
      rinv_arrs : per-core float32 [128, pad] (1/max(deg,1) replicated)
    """
    T = pad // P
    groups = make_groups(T)
    core = dst // sh
    loc = dst % sh
    tl = loc // P
    dloc = loc % P
    ishi = (src_g >= half).astype(np.int64)

    key = (core * T + tl) * 2 + ishi
    order = np.argsort(key, kind="stable")
    key_s = key[order]
    src_s = (src_g - ishi * half)[order]
    dloc_s = dloc[order]

    counts = np.bincount(key, minlength=ncores * T * 2).reshape(ncores, T, 2)
    kl = _cdiv(counts[:, :, 0], P).max(axis=0)  # [T]
    kh = _cdiv(counts[:, :, 1], P).max(axis=0)

    # chunk positions
    pos = np.zeros((T, 2), np.int64)
    ginfo = []
    c = 0
    for (t0, nt) in groups:
        gi = {"c0": int(c), "nlo": int(kl[t0:t0 + nt].sum()),
              "nhi": int(kh[t0:t0 + nt].sum()), "tiles": []}
        run = c
        for t in range(t0, t0 + nt):
            pos[t, 0] = run
            run += kl[t]
        for t in range(t0, t0 + nt):
            pos[t, 1] = run
            run += kh[t]
        for t in range(t0, t0 + nt):
            gi["tiles"].append((int(t), int(kl[t]), int(kh[t]),
                                int(pos[t, 0]), int(pos[t, 1])))
        c = run
        ginfo.append(gi)
    TC = int(c)

    group_start = np.concatenate([[0], np.cumsum(counts.transpose(0, 1, 2)
                                                 .ravel())])[:-1]
    rank = np.arange(len(dst)) - group_start[key_s]

    idx16_arrs, dst_arrs, rinv_arrs = [], [], []
    deg = np.bincount(dst, minlength=sh * ncores).astype(np.float32)
    inv = 1.0 / np.maximum(deg, 1.0)
    pos_flat = pos.ravel()  # [(t,h)] -> chunk pos
    for cc in range(ncores):
        sel = (key_s // (2 * T)) == cc
        th = key_s[sel] % (2 * T)          # t*2 + h
        r_c = rank[sel]
        col = pos_flat[th] + r_c // P
        row = r_c % P
        idx_flat = np.zeros((TC * P,), np.int32)
        da = np.full((P, TC), -1.0, np.float32)  # cast to bf16 below
        idx_flat[col * P + row] = src_s[sel]
        da[row, col] = dloc_s[sel]
        # dma_gather wrap: edge j -> [16*rep + j%16, j//16]
        i16 = np.ascontiguousarray(
            np.tile(idx_flat.reshape(TC * 8, 16).T.astype(np.int16), (8, 1)))
        idx16_arrs.append(i16)
        dst_arrs.append(da.astype(ml_dtypes.bfloat16))
        rv = np.zeros((pad,), np.float32)
        rv[:sh] = inv[cc * sh:(cc + 1) * sh]
        rinv_arrs.append(np.ascontiguousarray(
            np.broadcast_to(rv.astype(ml_dtypes.bfloat16), (P, pad))))
    return ginfo, TC, idx16_arrs, dst_arrs, rinv_arrs


def _remap(idx, sh, pad):
    """global node id -> index in the quartered-AllGather gather layout.

    The row-major per-core buffer [pad, D] is AllGathered in 4 row-quarters
    (QR = pad//4 rows each); quarter q's collective output occupies rows
    [q*8*QR, (q+1)*8*QR) of the gather table, rank-major within.
    """
    qr = pad // 4
    core = idx // sh
    local = idx % sh
    q = local // qr
    lq = local % qr
    return q * (8 * qr) + core * qr + lq


# ---------------------------------------------------------------------------
# Device program
# ---------------------------------------------------------------------------

def _bcast_chunks(ap_2d, extra, axis_inner):
    """Append / interleave a broadcast dim on a 2-D SBUF AP.

    axis_inner=True : [p, k] -> [p, k, B(extra)]  (value repeated along new
                      innermost dim)
    axis_inner=False: [p, n] -> [p, B(extra), n]
    """
    base = ap_2d
    if axis_inner:
        new = [list(base.ap[0]), list(base.ap[1]), [0, extra]]
    else:
        new = [list(base.ap[0]), [0, extra], list(base.ap[1])]
    return bass.AP(base.tensor, base.offset, new)


class Builder:
    def __init__(self, plan):
        self.plan = plan
        nc = bacc.Bacc(
            "TRN2",
            target_bir_lowering=False,
            debug=False,
            num_devices=plan["ncores"],
            num_swdge_queues=4,
        )
        self._gq = 0
        self.nc = nc
        self.pad = plan["pad"]          # nodes per core per class (mult of 128)
        self.T = self.pad // P          # dst tiles per class
        self.gpad = plan["ncores"] * self.pad

        # ---- I/O tensors -------------------------------------------------
        dram = nc.dram_tensor
        self.t_cell_lo = dram("cell_lo_gather", [self.gpad, D], FEAT_DT,
                              kind="ExternalInput")
        self.t_netT = dram("netT", [D, self.pad], FEAT_DT, kind="ExternalInput")
        self.t_cellT = dram("cellT", [D, 2 * self.pad], FEAT_DT,
                            kind="ExternalInput")
        self.t_w = {}
        for m in ("c2n", "n2c", "c2c"):
            self.t_w[m, "w1"] = dram(f"{m}_w1", [2 * D, D], FEAT_DT,
                                     kind="ExternalInput")
            self.t_w[m, "w2"] = dram(f"{m}_w2", [D, D], FEAT_DT,
                                     kind="ExternalInput")
            self.t_w[m, "b1"] = dram(f"{m}_b1", [P, 2], F32,
                                     kind="ExternalInput")
            self.t_w[m, "b2"] = dram(f"{m}_b2", [P, 2], F32,
                                     kind="ExternalInput")
        self.t_edge = {}
        for a in ("c2n", "n2c", "c2c"):
            TC = max(plan["TC"][a], 1)
            self.t_edge[a, "idx"] = dram(f"{a}_idx", [P, TC * 8],
                                         mybir.dt.int16, kind="ExternalInput")
            self.t_edge[a, "dst"] = dram(f"{a}_dst", [P, TC], FEAT_DT,
                                         kind="ExternalInput")
            self.t_edge[a, "rinv"] = dram(f"{a}_rinv", [P, self.pad],
                                          FEAT_DT, kind="ExternalInput")
        if not plan["net_ln_trivial"]:
            self.t_net_g = dram("net_ln_g_b", [P, 2 * D], F32,
                                kind="ExternalInput")
        if not plan["cell_ln_trivial"]:
            self.t_cell_g = dram("cell_ln_g_b", [P, 2 * D], F32,
                                 kind="ExternalInput")

        self.t_net_out = dram("net_out", [self.pad, D], F32,
                              kind="ExternalOutput")
        self.t_cell_out = dram("cell_out", [2 * self.pad, D], F32,
                               kind="ExternalOutput")

        # internal DRAM
        self.t_net_rm = dram("net_rm", [self.pad, D], FEAT_DT, kind="Internal")
        self.t_net_ag = dram("net_ag", [self.gpad, D], FEAT_DT,
                             kind="Internal", addr_space="Shared")
        self.t_cell_rm = dram("cell_rm", [self.pad, D], FEAT_DT,
                              kind="Internal")
        self.t_cell_ag = dram("cell_ag", [self.gpad, D], FEAT_DT,
                              kind="Internal", addr_space="Shared")
        self.t_cell2T = dram("cell2T", [D, 2 * self.pad], FEAT_DT,
                             kind="Internal")

    # -- helpers -----------------------------------------------------------

    def groups(self):
        return make_groups(self.T)

    def load_weights(self, wpool):
        nc = self.nc
        self.w_sb = {}
        self.wpool = wpool
        for m in ("c2n", "n2c", "c2c"):
            w1 = self.wpool.tile([P, 4 * D], FEAT_DT, name=f"w1_{m}")
            for c in range(4):
                nc.sync.dma_start(out=w1[:, c * D:(c + 1) * D],
                                  in_=self.t_w[m, "w1"][c * P:(c + 1) * P, :])
            w2 = self.wpool.tile([P, 2 * D], FEAT_DT, name=f"w2_{m}")
            for c in range(2):
                nc.sync.dma_start(out=w2[:, c * D:(c + 1) * D],
                                  in_=self.t_w[m, "w2"][c * P:(c + 1) * P, :])
            b1 = self.wpool.tile([P, 2], F32, name=f"b1_{m}")
            nc.sync.dma_start(out=b1[:], in_=self.t_w[m, "b1"][:])
            b2 = self.wpool.tile([P, 2], F32, name=f"b2_{m}")
            nc.sync.dma_start(out=b2[:], in_=self.t_w[m, "b2"][:])
            self.w_sb[m] = (w1, w2, b1, b2)

        it_i32 = self.wpool.tile([P, P], I32, name="iota_i")
        nc.gpsimd.iota(it_i32[:], pattern=[[1, P]], base=0,
                       channel_multiplier=0)
        self.iota_f = self.wpool.tile([P, P], FEAT_DT, name="iota_f")
        nc.vector.tensor_copy(out=self.iota_f[:], in_=it_i32[:])
        self.ident = self.wpool.tile([P, P], FEAT_DT, name="ident")
        make_identity(nc, self.ident[:])
        self.eps_t = self.wpool.tile([P, 1], F32, name="eps_t")
        nc.vector.memset(self.eps_t[:], EPS)
        self.lngb = {}
        if not self.plan["net_ln_trivial"]:
            g = self.wpool.tile([P, 2 * D], F32, name="net_gb")
            nc.sync.dma_start(out=g[:], in_=self.t_net_g[:])
            self.lngb["net"] = g
        if not self.plan["cell_ln_trivial"]:
            g = self.wpool.tile([P, 2 * D], F32, name="cell_gb")
            nc.sync.dma_start(out=g[:], in_=self.t_cell_g[:])
            self.lngb["cell"] = g

    def load_edge_meta(self, tc, a):
        """Load idx/dst/rinv for aggregation `a` into SBUF."""
        nc = self.nc
        TC = max(self.plan["TC"][a], 1)
        idx = self.epool.tile([P, TC * 8], mybir.dt.int16, tag="idx",
                              bufs=2, name=f"idx_{a}")
        nc.sync.dma_start(out=idx[:], in_=self.t_edge[a, "idx"][:])
        dstv = self.epool.tile([P, TC], FEAT_DT, tag="dst", bufs=2,
                               name=f"dst_{a}")
        nc.sync.dma_start(out=dstv[:], in_=self.t_edge[a, "dst"][:])
        rinv = self.epool.tile([P, self.pad], FEAT_DT, tag="rinv", bufs=1,
                               name=f"rinv_{a}")
        nc.sync.dma_start(out=rinv[:], in_=self.t_edge[a, "rinv"][:])
        return idx, dstv, rinv

    def aggregate_group(self, a, meta, src_dram, gi, nt):
        """Compute degree-normalized messages (feature-major) for the dst
        tiles of group `gi`. Returns (msg_lo, msg_hi) SBUF [128, nt*128]."""
        nc = self.nc
        idx, dstv, rinv = meta
        c0, nlo, nhi = gi["c0"], gi["nlo"], gi["nhi"]
        kg = nlo + nhi
        half = self.plan["half"]
        R = nt * P

        msg_lo = self.msgsb.tile([P, R], FEAT_DT, tag="msg_lo", name="msg_lo")
        msg_hi = self.msgsb.tile([P, R], FEAT_DT, tag="msg_hi", name="msg_hi")
        if kg == 0:
            nc.vector.memset(msg_lo[:], 0.0)
            nc.vector.memset(msg_hi[:], 0.0)
            return msg_lo, msg_hi

        # single_packet=False: a packet holds <=64 descriptors, so large
        # calls must not coalesce into one packet. Calls capped at 28 chunks.
        GMAX = 28
        F = self.fpool.tile([P, kg, D], FEAT_DT, tag="gather", name="Fg")
        for (base, cnt, src) in ((0, nlo, src_dram[:]),
                                 (nlo, nhi, src_dram[half:, :])):
            for s in range(base, base + cnt, GMAX):
                nn = min(GMAX, base + cnt - s)
                n = nn * P
                nc.gpsimd.dma_gather(F[:, s:s + nn, :], src,
                                     idx[:, (c0 + s) * 8:(c0 + s + nn) * 8],
                                     n, n, D, queue_num=self._gq,
                                     single_packet=False)
                self._gq = (self._gq + 1) % NQ

        ps_lo = self.mpsum.tile([P, R], F32, tag="ps_lo", name="ps_lo")
        ps_hi = self.mpsum.tile([P, R], F32, tag="ps_hi", name="ps_hi")

        for ti, (t, kl, kh, plo, phi) in enumerate(gi["tiles"]):
            k = kl + kh
            osl = slice(ti * P, (ti + 1) * P)
            if k == 0:
                nc.vector.memset(msg_lo[:, osl], 0.0)
                nc.vector.memset(msg_hi[:, osl], 0.0)
                continue
            done = 0
            for (p0, kk) in ((plo, kl), (phi, kh)):
                if kk == 0:
                    continue
                S = self.spool.tile([P, kk * P], FEAT_DT, tag="sel", name="S")
                in0 = _bcast_chunks(self.iota_f[:], kk, axis_inner=False)
                in1 = _bcast_chunks(dstv[:, p0:p0 + kk], P, axis_inner=True)
                nc.vector.tensor_tensor(out=S[:], in0=in0, in1=in1,
                                        op=mybir.AluOpType.is_equal)
                for j in range(kk):
                    rel = p0 - c0 + j
                    nc.tensor.matmul(out=ps_lo[:, osl],
                                     lhsT=F[:, rel, 0:DH],
                                     rhs=S[:, j * P:(j + 1) * P],
                                     start=(done + j == 0),
                                     stop=(done + j == k - 1))
                    nc.tensor.matmul(out=ps_hi[:, osl],
                                     lhsT=F[:, rel, DH:D],
                                     rhs=S[:, j * P:(j + 1) * P],
                                     start=(done + j == 0),
                                     stop=(done + j == k - 1))
                done += kk
            # normalize by 1/deg while copying PSUM -> SBUF
            rs = rinv[:, t * P:(t + 1) * P]
            nc.vector.tensor_tensor(out=msg_lo[:, osl], in0=ps_lo[:, osl],
                                    in1=rs, op=mybir.AluOpType.mult)
            nc.vector.tensor_tensor(out=msg_hi[:, osl], in0=ps_hi[:, osl],
                                    in1=rs, op=mybir.AluOpType.mult)
        return msg_lo, msg_hi

    def mlp(self, m, h_lo, h_hi, msg, R):
        """Feature-major 2-layer MLP + residual.

        h_lo/h_hi: [128, R] SBUF tiles (input features, chunks 0/1)
        msg: None or (msg_lo, msg_hi)
        Returns (z0, z1) SBUF tiles [128, R] (feat chunks of the residual sum).
        """
        nc = self.nc
        w1, w2, b1, b2 = self.w_sb[m]
        chunks = [h_lo, h_hi] + ([msg[0], msg[1]] if msg is not None else [])
        ys = []
        for mi in range(2):
            o1 = self.mlppsum.tile([P, R], F32, tag="mm", name=f"o1_{mi}")
            for c, rt in enumerate(chunks):
                nc.tensor.matmul(out=o1[:],
                                 lhsT=w1[:, c * D + mi * DH:c * D + (mi + 1) * DH],
                                 rhs=rt[:],
                                 start=(c == 0), stop=(c == len(chunks) - 1))
            y = self.ypool.tile([P, R], FEAT_DT, tag="y1", name=f"y1_{mi}")
            nc.scalar.activation(out=y[:], in_=o1[:],
                                 func=mybir.ActivationFunctionType.Relu,
                                 bias=b1[:, mi:mi + 1], scale=1.0)
            ys.append(y)
        zs = []
        for mi in range(2):
            o2 = self.mlppsum.tile([P, R], F32, tag="mm", name=f"o2_{mi}")
            for c in range(2):
                nc.tensor.matmul(out=o2[:],
                                 lhsT=w2[:, c * D + mi * DH:c * D + (mi + 1) * DH],
                                 rhs=ys[c][:],
                                 start=(c == 0), stop=(c == 1))
            # bias + residual: z = (o2 + b2) + h
            zb = self.zpool.tile([P, R], FEAT_DT, tag="zb", bufs=2,
                                 name=f"zb_{mi}")
            nc.scalar.activation(out=zb[:], in_=o2[:],
                                 func=mybir.ActivationFunctionType.Identity,
                                 bias=b2[:, mi:mi + 1], scale=1.0)
            z = self.zpool.tile([P, R], FEAT_DT, tag="z", bufs=4,
                                 name=f"zr_{mi}")
            nc.vector.tensor_tensor(out=z[:], in0=zb[:],
                                    in1=(h_lo if mi == 0 else h_hi)[:],
                                    op=mybir.AluOpType.add)
            zs.append(z)
        return zs

    def row_major_tiles(self, zs, nt):
        """Transpose feature-major z (2x [128, nt*128]) into row-major
        [128, 256] tiles; yields (rt_index, row_tile)."""
        nc = self.nc
        for rt in range(nt):
            tp0 = self.tpsum.tile([P, P], FEAT_DT, tag="tp", name="tp0")
            tp1 = self.tpsum.tile([P, P], FEAT_DT, tag="tp", name="tp1")
            nc.tensor.transpose(out=tp0[:], in_=zs[0][:, rt * P:(rt + 1) * P],
                                identity=self.ident[:])
            nc.tensor.transpose(out=tp1[:], in_=zs[1][:, rt * P:(rt + 1) * P],
                                identity=self.ident[:])
            row = self.rowpool.tile([P, D], FEAT_DT, tag="row", name="row")
            nc.any.tensor_copy(out=row[:, :DH], in_=tp0[:])
            nc.any.tensor_copy(out=row[:, DH:], in_=tp1[:])
            yield rt, row

    def layernorm_row(self, row, which):
        """LayerNorm along free dim of row-major [128, 256] tile -> f32."""
        nc = self.nc
        st = self.lnpool.tile([P, 6], F32, tag="ln6", name="st6")
        nc.vector.bn_stats(out=st[:], in_=row[:])
        mv = self.lnpool.tile([P, 2], F32, tag="ln2", name="mv")
        nc.vector.bn_aggr(out=mv[:], in_=st[:])
        std = self.lnpool.tile([P, 1], F32, tag="ln1a", name="std")
        nc.scalar.activation(out=std[:], in_=mv[:, 1:2],
                             func=mybir.ActivationFunctionType.Sqrt,
                             bias=self.eps_t[:, 0:1], scale=1.0)
        rstd = self.lnpool.tile([P, 1], F32, tag="ln1b", name="rstd")
        nc.vector.reciprocal(out=rstd[:], in_=std[:])
        mb = self.lnpool.tile([P, 1], F32, tag="ln1c", name="mb")
        nc.vector.tensor_scalar(mb[:], mv[:, 0:1], rstd[:, 0:1], -1.0,
                                mybir.AluOpType.mult,
                                mybir.AluOpType.mult)
        out = self.orowpool.tile([P, D], F32, tag="orow", name="orow")
        nc.scalar.activation(out=out[:], in_=row[:],
                             func=mybir.ActivationFunctionType.Identity,
                             bias=mb[:, 0:1], scale=rstd[:, 0:1])
        gb = self.lngb.get(which)
        if gb is not None:
            nc.vector.tensor_tensor(out=out[:], in0=out[:], in1=gb[:, :D],
                                    op=mybir.AluOpType.mult)
            nc.vector.tensor_tensor(out=out[:], in0=out[:], in1=gb[:, D:],
                                    op=mybir.AluOpType.add)
        return out

    # -- full program --------------------------------------------------------

    def build(self):
        nc = self.nc
        plan = self.plan
        rg = [list(range(plan["ncores"]))]
        with tile.TileContext(nc) as tc:
            with tc.tile_pool(name="edges", bufs=1) as epool, \
                 tc.tile_pool(name="gather", bufs=2) as fpool, \
                 tc.tile_pool(name="sel", bufs=4) as spool, \
                 tc.tile_pool(name="msgps", bufs=1, space="PSUM") as mpsum, \
                 tc.tile_pool(name="mlpps", bufs=4, space="PSUM") as mlppsum, \
                 tc.tile_pool(name="tpps", bufs=2, space="PSUM") as tpsum, \
                 tc.tile_pool(name="msgsb", bufs=2) as msgsb, \
                 tc.tile_pool(name="hpool", bufs=2) as hpool, \
                 tc.tile_pool(name="ypool", bufs=3) as ypool, \
                 tc.tile_pool(name="zpool", bufs=3) as zpool, \
                 tc.tile_pool(name="rows", bufs=4) as rowpool, \
                 tc.tile_pool(name="orows", bufs=4) as orowpool, \
                 tc.tile_pool(name="ln", bufs=4) as lnpool, \
                 tc.tile_pool(name="weights", bufs=1) as wpool:
                self.epool = epool
                self.fpool = fpool
                self.spool = spool
                self.mpsum = mpsum
                self.mlppsum = mlppsum
                self.tpsum = tpsum
                self.msgsb = msgsb
                self.hpool = hpool
                self.ypool = ypool
                self.zpool = zpool
                self.rowpool = rowpool
                self.orowpool = orowpool
                self.lnpool = lnpool

                self.load_weights(wpool)
                nc.gpsimd.load_library(MLP_LIB)

                # ---------------- Phase 1: cells -> nets -------------------
                meta = self.load_edge_meta(tc, "c2n")
                for gi, (t0, nt) in zip(self.plan["ginfo"]["c2n"],
                                        self.groups()):
                    R = nt * P
                    col = t0 * P
                    msg = self.aggregate_group("c2n", meta, self.t_cell_lo,
                                               gi, nt)
                    h_lo = hpool.tile([P, R], FEAT_DT, tag="h_lo", name="h_lo")
                    h_hi = hpool.tile([P, R], FEAT_DT, tag="h_hi", name="h_hi")
                    nc.sync.dma_start(out=h_lo[:],
                                      in_=self.t_netT[0:DH, col:col + R])
                    nc.sync.dma_start(out=h_hi[:],
                                      in_=self.t_netT[DH:D, col:col + R])
                    zs = self.mlp("c2n", h_lo, h_hi, msg, R)
                    for rt, row in self.row_major_tiles(zs, nt):
                        orow = self.layernorm_row(row, "net")
                        r0 = col + rt * P
                        nc.sync.dma_start(out=self.t_net_out[r0:r0 + P, :],
                                          in_=orow[:])
                        rowb = self.rowpool.tile([P, D], FEAT_DT, tag="rowb",
                                                 bufs=4, name="rowb")
                        nc.scalar.copy(out=rowb[:], in_=orow[:])
                        nc.sync.dma_start(out=self.t_net_rm[r0:r0 + P, :],
                                          in_=rowb[:])

                QR = self.pad // 4
                for q in range(4):
                    nc.gpsimd.collective_compute(
                        "AllGather", mybir.AluOpType.bypass,
                        replica_groups=rg,
                        ins=[self.t_net_rm.ap()[q * QR:(q + 1) * QR, :]],
                        outs=[self.t_net_ag.ap()[q * 8 * QR:(q + 1) * 8 * QR, :]])

                # ---------------- Phase 2: nets -> cells -------------------
                meta = self.load_edge_meta(tc, "n2c")
                # high cells first (no aggregation; overlaps the AllGather)
                for (t0, nt) in self.groups():
                    R = nt * P
                    col = self.pad + t0 * P
                    h_lo = hpool.tile([P, R], FEAT_DT, tag="h_lo", name="h_lo")
                    h_hi = hpool.tile([P, R], FEAT_DT, tag="h_hi", name="h_hi")
                    nc.sync.dma_start(out=h_lo[:],
                                      in_=self.t_cellT[0:DH, col:col + R])
                    nc.sync.dma_start(out=h_hi[:],
                                      in_=self.t_cellT[DH:D, col:col + R])
                    zs = self.mlp("n2c", h_lo, h_hi, None, R)
                    nc.sync.dma_start(out=self.t_cell2T[0:DH, col:col + R],
                                      in_=zs[0][:])
                    nc.sync.dma_start(out=self.t_cell2T[DH:D, col:col + R],
                                      in_=zs[1][:])
                # low cells
                for gi, (t0, nt) in zip(self.plan["ginfo"]["n2c"],
                                        self.groups()):
                    R = nt * P
                    col = t0 * P
                    msg = self.aggregate_group("n2c", meta,
                                               self.t_net_ag, gi, nt)
                    h_lo = hpool.tile([P, R], FEAT_DT, tag="h_lo", name="h_lo")
                    h_hi = hpool.tile([P, R], FEAT_DT, tag="h_hi", name="h_hi")
                    nc.sync.dma_start(out=h_lo[:],
                                      in_=self.t_cellT[0:DH, col:col + R])
                    nc.sync.dma_start(out=h_hi[:],
                                      in_=self.t_cellT[DH:D, col:col + R])
                    zs = self.mlp("n2c", h_lo, h_hi, msg, R)
                    nc.sync.dma_start(out=self.t_cell2T[0:DH, col:col + R],
                                      in_=zs[0][:])
                    nc.sync.dma_start(out=self.t_cell2T[DH:D, col:col + R],
                                      in_=zs[1][:])
                    for rt, row in self.row_major_tiles(zs, nt):
                        r0 = col + rt * P
                        nc.sync.dma_start(out=self.t_cell_rm[r0:r0 + P, :],
                                          in_=row[:])

                for q in range(4):
                    nc.gpsimd.collective_compute(
                        "AllGather", mybir.AluOpType.bypass,
                        replica_groups=rg,
                        ins=[self.t_cell_rm.ap()[q * QR:(q + 1) * QR, :]],
                        outs=[self.t_cell_ag.ap()[q * 8 * QR:(q + 1) * 8 * QR, :]])

                # ---------------- Phase 3: cells -> cells ------------------
                meta = self.load_edge_meta(tc, "c2c")
                # high cells (no aggregation; overlaps the AllGather)
                for (t0, nt) in self.groups():
                    R = nt * P
                    col = self.pad + t0 * P
                    h_lo = hpool.tile([P, R], FEAT_DT, tag="h_lo", name="h_lo")
                    h_hi = hpool.tile([P, R], FEAT_DT, tag="h_hi", name="h_hi")
                    nc.sync.dma_start(out=h_lo[:],
                                      in_=self.t_cell2T[0:DH, col:col + R])
                    nc.sync.dma_start(out=h_hi[:],
                                      in_=self.t_cell2T[DH:D, col:col + R])
                    zs = self.mlp("c2c", h_lo, h_hi, None, R)
                    for rt, row in self.row_major_tiles(zs, nt):
                        orow = self.layernorm_row(row, "cell")
                        r0 = col + rt * P
                        nc.sync.dma_start(out=self.t_cell_out[r0:r0 + P, :],
                                          in_=orow[:])
                # low cells
                for gi, (t0, nt) in zip(self.plan["ginfo"]["c2c"],
                                        self.groups()):
                    R = nt * P
                    col = t0 * P
                    msg = self.aggregate_group("c2c", meta,
                                               self.t_cell_ag, gi, nt)
                    h_lo = hpool.tile([P, R], FEAT_DT, tag="h_lo", name="h_lo")
                    h_hi = hpool.tile([P, R], FEAT_DT, tag="h_hi", name="h_hi")
                    nc.sync.dma_start(out=h_lo[:],
                                      in_=self.t_cell2T[0:DH, col:col + R])
                    nc.sync.dma_start(out=h_hi[:],
                                      in_=self.t_cell2T[DH:D, col:col + R])
                    zs = self.mlp("c2c", h_lo, h_hi, msg, R)
                    for rt, row in self.row_major_tiles(zs, nt):
                        orow = self.layernorm_row(row, "cell")
                        r0 = col + rt * P
                        nc.sync.dma_start(out=self.t_cell_out[r0:r0 + P, :],
                                          in_=orow[:])

        nc.compile()
        return nc


# ---------------------------------------------------------------------------
# Host orchestration
# ---------------------------------------------------------------------------

_CACHE = {}


def prepare(inputs, n_cell, n_net, ncores=NCORES):
    sh = n_net // ncores            # nodes per core per class
    pad = _cdiv(sh, P) * P
    assert n_cell == 2 * n_net

    cell_h = np.asarray(inputs["cell_h"], np.float32)
    net_h = np.asarray(inputs["net_h"], np.float32)

    e_c2n = np.asarray(inputs["cell_to_net_edge_index"]).astype(np.int64)
    e_n2c = np.asarray(inputs["net_to_cell_edge_index"]).astype(np.int64)
    e_c2c = np.asarray(inputs["cell_to_cell_edge_index"]).astype(np.int64)

    half = (ncores // 2) * pad
    plan = {"ncores": ncores, "pad": pad, "half": half, "ginfo": {},
            "TC": {}}
    edge_data = {}
    for name, (src, dst) in (("c2n", (e_c2n[0], e_c2n[1])),
                             ("n2c", (e_n2c[0], e_n2c[1])),
                             ("c2c", (e_c2c[0], e_c2c[1]))):
        gf, TC, ia, da, rv = preprocess_edges(_remap(src, sh, pad), dst, sh,
                                              pad, ncores, half)
        plan["ginfo"][name] = gf
        plan["TC"][name] = TC
        edge_data[name] = (ia, da, rv)

    net_g = np.asarray(inputs["net_ln_g"], np.float32)
    net_b = np.asarray(inputs["net_ln_b"], np.float32)
    cell_g = np.asarray(inputs["cell_ln_g"], np.float32)
    cell_b = np.asarray(inputs["cell_ln_b"], np.float32)
    plan["net_ln_trivial"] = bool(np.all(net_g == 1.0) and np.all(net_b == 0.0))
    plan["cell_ln_trivial"] = bool(np.all(cell_g == 1.0)
                                   and np.all(cell_b == 0.0))

    # replicated gather source: original low cells, padded-concat layout
    gpad = ncores * pad
    qr = pad // 4
    cell_lo = np.zeros((gpad, D), FEAT_NP)
    for c in range(ncores):
        for q in range(4):
            n = max(0, min(sh - q * qr, qr))
            if n:
                cell_lo[q * 8 * qr + c * qr:q * 8 * qr + c * qr + n] = \
                    cell_h[c * sh + q * qr:c * sh + q * qr + n]

    in_maps = []
    for c in range(ncores):
        m = {"cell_lo_gather": cell_lo}
        netT = np.zeros((D, pad), FEAT_NP)
        netT[:, :sh] = net_h[c * sh:(c + 1) * sh].T
        m["netT"] = netT
        cellT = np.zeros((D, 2 * pad), FEAT_NP)
        cellT[:, :sh] = cell_h[c * sh:(c + 1) * sh].T
        cellT[:, pad:pad + sh] = cell_h[n_net + c * sh:n_net + (c + 1) * sh].T
        m["cellT"] = cellT
        for mm in ("c2n", "n2c", "c2c"):
            m[f"{mm}_w1"] = np.asarray(inputs[f"{mm}_w1"], FEAT_NP)
            m[f"{mm}_w2"] = np.asarray(inputs[f"{mm}_w2"], FEAT_NP)
            m[f"{mm}_b1"] = np.ascontiguousarray(
                np.asarray(inputs[f"{mm}_b1"], np.float32).reshape(2, P).T)
            m[f"{mm}_b2"] = np.ascontiguousarray(
                np.asarray(inputs[f"{mm}_b2"], np.float32).reshape(2, P).T)
        for a in ("c2n", "n2c", "c2c"):
            ia, da, rv = edge_data[a]
            m[f"{a}_idx"] = (ia[c] if ia[c].shape[1]
                             else np.zeros((P, 8), np.int16))
            m[f"{a}_dst"] = (da[c] if da[c].shape[1]
                             else np.full((P, 1), -1.0, np.float32))
            m[f"{a}_rinv"] = rv[c]
        if not plan["net_ln_trivial"]:
            m["net_ln_g_b"] = np.ascontiguousarray(np.broadcast_to(
                np.concatenate([net_g, net_b]), (P, 2 * D)).astype(np.float32))
        if not plan["cell_ln_trivial"]:
            m["cell_ln_g_b"] = np.ascontiguousarray(np.broadcast_to(
                np.concatenate([cell_g, cell_b]), (P, 2 * D)).astype(np.float32))
        in_maps.append(m)
    return plan, in_maps, sh, pad


def assemble(results, n_cell, n_net, sh, pad, ncores=NCORES):
    cell = np.empty((n_cell, D), np.float32)
    net = np.empty((n_net, D), np.float32)
    for c in range(ncores):
        co = results[c]["cell_out"]
        cell[c * sh:(c + 1) * sh] = co[:sh]
        cell[n_net + c * sh:n_net + (c + 1) * sh] = co[pad:pad + sh]
        net[c * sh:(c + 1) * sh] = results[c]["net_out"][:sh]
    return cell, net


def get_program(plan):
    def gkey(a):
        return tuple((g["c0"], g["nlo"], g["nhi"], tuple(g["tiles"]))
                     for g in plan["ginfo"][a])
    key = (tuple(gkey(a) for a in ("c2n", "n2c", "c2c")),
           plan["pad"], plan["ncores"], plan["net_ln_trivial"],
           plan["cell_ln_trivial"])
    if key not in _CACHE:
        _CACHE[key] = Builder(plan).build()
    return _CACHE[key]


def kernel(**inputs):
    n_cell = inputs["cell_h"].shape[0]
    n_net = inputs["net_h"].shape[0]
    plan, in_maps, sh, pad = prepare(inputs, n_cell, n_net)
    nc = get_program(plan)
    res = bass_utils.run_bass_kernel_spmd(nc, in_maps,
                                          core_ids=list(range(NCORES)))
    return assemble(res.results, n_cell, n_net, sh, pad)
